# revision 11
# baseline (speedup 1.0000x reference)
"""BloomAttention Trainium2 kernel (v2).

Reference semantics (B=2, S=2048, H=2048, NH=16, HD=128):
  mixed = hs @ w_qkv.T + b_qkv, reshaped [b,s,nh,3hd] then reinterpreted
  Megatron-style as (s, b*nh, hd).  With B=2 that reinterpretation scrambles
  (batch, position) into 32 independent "virtual sequences" indexed by
  (parity p, head n): virtual seq (p, n) consists of flat tokens
  t = 2*s' + p (t = b*S + s_pos) in increasing s' order.  Attention (with
  alibi[n, k'] bias, causal mask over virtual positions, softmax) runs per
  virtual sequence; the dense projection maps back so that
  out[p, s', :] = dense(concat_n ctx_{p,n}[s']).

Sharding: 2 heads per core (Megatron column-split of w_qkv, row-split of
w_dense), both parities; host sums the 8 partial dense outputs.

Device layouts (per core c, heads {2c, 2c+1}):
  hsr  [2048h, 4096t']   t' = p*2048 + s'  (host pre-transposed/reordered)
  qk   [128j, 1024t']    packed per (jt, p, half) strips; j = q/k head dims
  v    [4096t', 256c']   c' = (n_l, d)
  scores S.T [k', q'] strips of 1024 queries; P = exp(S/sqrt(HD)+alibi)*tri
  den: DVE bf16 tile-accumulate over k-tiles + GpSimd partition_all_reduce
  ctx.T [128d, q'] accumulated in PSUM per 512-query block
  dense: lagged one block, matmuls interleaved into the attention PE
  stream; PSUM->SBUF copies rotate DVE/Pool/Scalar; DMA out on sync.
"""

import math
import os
import sys

for _p in ("/opt/trn_rl_repo", "/root/.axon_site/_ro/trn_rl_repo"):
    if os.path.isdir(_p) and _p not in sys.path:
        sys.path.append(_p)

import numpy as np
import ml_dtypes
import concourse.bass as bass
import concourse.tile as tile
from concourse import mybir, bacc, bass_isa
from concourse.bass_utils import run_bass_kernel_spmd

F32 = mybir.dt.float32
BF16 = mybir.dt.bfloat16
AF = mybir.ActivationFunctionType

B, S, H, NH = 2, 2048, 2048, 16
HD = H // NH
T = B * S                  # 4096 flat tokens
NHT = H // 128             # 16 h-tiles
JQK = 4 * 128              # local q+k rows
JV = 2 * 128               # local v rows
NTB = T // 512             # 8 token-blocks
NKT = S // 128             # 16 key tiles per virtual sequence
NSB = S // 512             # 4 query blocks per virtual sequence
INV_SQRT_HD = 1.0 / math.sqrt(HD)

_cache = {}


def _build_nc():
    nc = bacc.Bacc()
    hsr = nc.declare_dram_parameter("hsr", [H, T], BF16, isOutput=False)
    wqkT = nc.declare_dram_parameter("wqkT", [H, JQK], BF16, isOutput=False)
    wvT = nc.declare_dram_parameter("wvT", [H, JV], BF16, isOutput=False)
    wdT = nc.declare_dram_parameter("wdT", [JV, H], BF16, isOutput=False)
    bqk = nc.declare_dram_parameter("bqk", [JQK], F32, isOutput=False)
    bvbc = nc.declare_dram_parameter("bvbc", [128, JV], F32, isOutput=False)
    albt = nc.declare_dram_parameter("albt", [128, 2, NKT], F32, isOutput=False)
    mskt = nc.declare_dram_parameter("mskt", [128, 896], BF16, isOutput=False)
    part = nc.declare_dram_parameter("part", [T, H], BF16, isOutput=True)

    with tile.TileContext(nc) as tc:
        with (
            tc.tile_pool(name="consts", bufs=1) as consts,
            tc.tile_pool(name="qkvout", bufs=1) as qkvout,
        ):
            # ---------------- Phase A: front-load DMAs ----------------
            # Emission order matters: the first matmul needs wqk(hg0,j0) and
            # hsr(tb0,hg0,j0) only, so those go first on their queues.
            with tc.tile_pool(name="wpool", bufs=1) as wpool:
                wqk_big = []
                wv_big = []
                for hg in range(4):
                    wqk_big.append(wpool.tile([128, 4, JQK], BF16,
                                              tag=f"wqk{hg}", name=f"wqk{hg}"))
                    wv_big.append(wpool.tile([128, 4, JV], BF16,
                                             tag=f"wv{hg}", name=f"wv{hg}"))
                # wqk hg0 split by j for lowest first-tile latency (sync
                # queue); j0 further split in two for parallel DMA engines
                nc.sync.dma_start(out=wqk_big[0][:, 0, 0:256],
                                  in_=wqkT[0:128, 0:256])
                nc.sync.dma_start(out=wqk_big[0][:, 0, 256:512],
                                  in_=wqkT[0:128, 256:512])
                for j in range(1, 4):
                    nc.sync.dma_start(
                        out=wqk_big[0][:, j, :],
                        in_=wqkT[j * 128:(j + 1) * 128, :])
                # hg1 on sync; hg2/hg3 on scalar queue
                nc.sync.dma_start(
                    out=wqk_big[1],
                    in_=wqkT[512:1024, :].rearrange("(j p) f -> p j f", p=128))
                for hg in (2, 3):
                    nc.scalar.dma_start(
                        out=wqk_big[hg],
                        in_=wqkT[hg * 512:(hg + 1) * 512, :].rearrange(
                            "(j p) f -> p j f", p=128))
                # wv split across sync/scalar queues
                for hg in range(4):
                    eng = nc.sync if hg % 2 == 0 else nc.scalar
                    eng.dma_start(
                        out=wv_big[hg],
                        in_=wvT[hg * 512:(hg + 1) * 512, :].rearrange(
                            "(j p) f -> p j f", p=128))
                # consts + dense weight, after the critical weights
                bqk_sb = consts.tile([128, 4], F32)
                nc.sync.dma_start(out=bqk_sb,
                                  in_=bqk.rearrange("(jt p) -> p jt", p=128))
                mask_sb = consts.tile([128, 896], BF16)
                nc.sync.dma_start(out=mask_sb, in_=mskt[:, :])
                tri128 = mask_sb[:, 384:512]
                ones128 = mask_sb[:, 512:640]
                bv_bc = consts.tile([128, JV], F32)
                nc.scalar.dma_start(out=bv_bc, in_=bvbc[:, :])
                alb_sb = consts.tile([128, 2, NKT], F32)
                nc.sync.dma_start(out=alb_sb, in_=albt[:, :, :])
                wd_sb = consts.tile([128, 2, H], BF16)
                for nl in range(2):
                    nc.scalar.dma_start(out=wd_sb[:, nl, :],
                                        in_=wdT[nl * 128:(nl + 1) * 128, :])

                def wqk_t(ht):
                    return wqk_big[ht // 4][:, ht % 4, :]

                def wv_tt(ht):
                    return wv_big[ht // 4][:, ht % 4, :]

                # qk strips packed per (jt, p, half): [128, 1024]
                qk_sb = {}
                for jt in range(4):
                    for p in range(2):
                        for hf in range(2):
                            qk_sb[(jt, p, hf)] = qkvout.tile(
                                [128, 1024], BF16, tag=f"qk{jt}_{p}{hf}",
                                name=f"qk{jt}_{p}{hf}")
                v_sb = {}   # tt -> [128, 256] tile

                # ---------------- Phase B: QKV projection ----------------
                with (
                    tc.tile_pool(name="hsrp", bufs=1) as hsrp,
                    tc.tile_pool(name="pqk", bufs=1, space="PSUM") as pqk,
                    tc.tile_pool(name="pvp", bufs=1, space="PSUM") as pvp,
                ):
                    # PE warm-up: dummy matmuls during the initial DMA wait so
                    # the tensor clock is fully ramped when real work arrives.
                    scratch = consts.tile([128, 512], BF16)
                    nc.vector.memset(scratch, 0.0)
                    pwarm = pqk.tile([128, 512], F32, tag="pq0", name="pwarm")
                    for _ in range(16):
                        nc.tensor.matmul(pwarm, lhsT=scratch[:, 0:128],
                                         rhs=scratch, start=True, stop=True)
                    for tb in range(NTB):
                        hbig = []
                        for hg in range(4):
                            h_t = hsrp.tile([128, 4, 512], BF16, tag="hsr",
                                            bufs=8, name=f"hsr{tb}_{hg}")
                            if tb == 0 and hg == 0:
                                nc.gpsimd.dma_start(out=h_t[:, 0, 0:256],
                                                    in_=hsr[0:128, 0:256])
                                nc.gpsimd.dma_start(out=h_t[:, 0, 256:512],
                                                    in_=hsr[0:128, 256:512])
                                for j in range(1, 4):
                                    nc.gpsimd.dma_start(
                                        out=h_t[:, j, :],
                                        in_=hsr[j * 128:(j + 1) * 128, 0:512])
                            else:
                                nc.gpsimd.dma_start(
                                    out=h_t,
                                    in_=hsr[hg * 512:(hg + 1) * 512,
                                            tb * 512:(tb + 1) * 512].rearrange(
                                                "(j p) f -> p j f", p=128))
                            hbig.append(h_t)

                        def hs_t(ht):
                            return hbig[ht // 4][:, ht % 4, :]

                        pq = [pqk.tile([128, 512], F32, tag=f"pq{jt}",
                                       name=f"pq{jt}_{tb}") for jt in range(4)]
                        pv = [pvp.tile([128, JV], F32, tag=f"pv{tt}",
                                       name=f"pv{tt}_{tb}") for tt in range(4)]
                        for ht in range(NHT):
                            st = ht == 0
                            sp = ht == NHT - 1
                            for jt in range(4):
                                nc.tensor.matmul(
                                    pq[jt],
                                    lhsT=wqk_t(ht)[:, jt * 128:(jt + 1) * 128],
                                    rhs=hs_t(ht),
                                    start=st, stop=sp,
                                )
                        for ht in range(NHT):
                            st = ht == 0
                            sp = ht == NHT - 1
                            for tt in range(4):
                                nc.tensor.matmul(
                                    pv[tt],
                                    lhsT=hs_t(ht)[:, tt * 128:(tt + 1) * 128],
                                    rhs=wv_tt(ht),
                                    start=st, stop=sp,
                                )
                        p_of, i_of = tb // 4, tb % 4
                        hf_of, sl_of = i_of // 2, i_of % 2
                        for jt in range(4):
                            nc.vector.tensor_scalar_add(
                                qk_sb[(jt, p_of, hf_of)][:, sl_of * 512:
                                                         (sl_of + 1) * 512],
                                pq[jt], bqk_sb[:, jt:jt + 1])
                        for tt in range(4):
                            vt = qkvout.tile([128, JV], BF16,
                                             tag=f"v{tb * 4 + tt}",
                                             name=f"v{tb * 4 + tt}")
                            nc.vector.tensor_add(vt, pv[tt], bv_bc)
                            v_sb[tb * 4 + tt] = vt

            # ---------------- Phase C: attention + dense ----------------
            with (
                tc.tile_pool(name="ctxp", bufs=1) as ctxp,
                tc.tile_pool(name="ptp", bufs=1) as ptp,
                tc.tile_pool(name="daccp", bufs=1) as daccp,
                tc.tile_pool(name="smallp", bufs=1) as smallp,
                tc.tile_pool(name="outsbp", bufs=1) as outsbp,
                tc.tile_pool(name="pstp", bufs=1, space="PSUM") as pstp,
                tc.tile_pool(name="pctxp", bufs=1, space="PSUM") as pctxp,
                tc.tile_pool(name="poutp", bufs=1, space="PSUM") as poutp,
            ):
                ctx_t = {}  # (p, b) -> [128, 2, 512] tile

                # ---- lagged dense machinery ----
                dense_fifo = []
                # copy-engine rotation: DVE x9, Scalar x7 per block
                # (GPSIMD cannot read PSUM)
                COPY_PAT = "VSVSVVSVSVVSVSVS"
                # out-DMA issue queue rotation: sync x10, gpsimd x6
                DMAQ_PAT = "YGYYGYYGYYGYYGYG"

                def push_dense(p, b, split_dma=False):
                    ct = ctx_t[(p, b)]
                    for idx in range(16):
                        i, hb = idx // 4, idx % 4
                        eng = COPY_PAT[idx]
                        dq = DMAQ_PAT[idx]

                        def chunk(i=i, hb=hb, eng=eng, dq=dq, ct=ct, p=p, b=b):
                            tt = p * 16 + b * 4 + i
                            po = poutp.tile([128, 512], F32, tag="pout",
                                            bufs=2, name=f"po{tt}_{hb}")
                            for nl in range(2):
                                nc.tensor.matmul(
                                    po,
                                    lhsT=ct[:, nl, i * 128:(i + 1) * 128],
                                    rhs=wd_sb[:, nl, hb * 512:(hb + 1) * 512],
                                    start=(nl == 0), stop=(nl == 1),
                                )
                            ot = outsbp.tile([128, 512], BF16, tag="outsb",
                                             bufs=12, name=f"ot{tt}_{hb}")
                            if eng == "V":
                                nc.vector.tensor_copy(out=ot, in_=po)
                            else:
                                nc.scalar.copy(out=ot, in_=po)
                            dst = part[tt * 128:(tt + 1) * 128,
                                       hb * 512:(hb + 1) * 512]
                            if split_dma:
                                nc.sync.dma_start(out=dst[:, 0:256],
                                                  in_=ot[:, 0:256])
                                nc.gpsimd.dma_start(out=dst[:, 256:512],
                                                    in_=ot[:, 256:512])
                            elif dq == "Y":
                                nc.sync.dma_start(out=dst, in_=ot)
                            else:
                                nc.gpsimd.dma_start(out=dst, in_=ot)
                        dense_fifo.append(chunk)

                def emit_dense(n):
                    for _ in range(min(n, len(dense_fifo))):
                        dense_fifo.pop(0)()

                # ---- attention over one (p, nl, half) strip ----
                def attn_half(p, nl, hf):
                    nkt = 8 * hf + 8
                    b0, b1 = 2 * hf, 2 * hf + 1
                    pctx = {
                        b: pctxp.tile([128, 512], F32, tag="pctx", bufs=2,
                                      name=f"pctx{p}{nl}{b}")
                        for b in (b0, b1)
                    }
                    dacc = daccp.tile([128, 1024], BF16, tag="dacc", bufs=2,
                                      name=f"dacc{p}{nl}{hf}")
                    q_rhs = qk_sb[(2 * nl, p, hf)]
                    kq = qk_sb[(2 * nl + 1, p, 0)], qk_sb[(2 * nl + 1, p, 1)]
                    pts = {}

                    def off_of(kt):
                        return max(0, 128 * kt - 1024 * hf)

                    def st_exp(kt):
                        off = off_of(kt)
                        pst = pstp.tile([128, 1024], F32, tag="pst", bufs=2,
                                        name=f"pst{p}{nl}{hf}_{kt}")
                        ktile = kq[kt // 8][:, (kt % 8) * 128:(kt % 8 + 1) * 128]
                        for blk in range(2):
                            lo = max(off, blk * 512)
                            hi = (blk + 1) * 512
                            if lo >= hi:
                                continue
                            nc.tensor.matmul(
                                pst[:, lo:hi], lhsT=ktile, rhs=q_rhs[:, lo:hi],
                                start=True, stop=True,
                            )
                        pt = ptp.tile([128, 1024], BF16, tag="pt", bufs=4,
                                      name=f"pt{p}{nl}{hf}_{kt}")
                        nc.scalar.activation(out=pt[:, off:], in_=pst[:, off:],
                                             func=AF.Exp,
                                             bias=alb_sb[:, nl, kt:kt + 1],
                                             scale=INV_SQRT_HD)
                        if kt >= 8 * hf:  # diagonal k-tile for this half
                            nc.vector.tensor_mul(
                                pt[:, off:off + 128], pt[:, off:off + 128],
                                tri128)
                        pts[kt] = pt

                    def pv_step(kt):
                        off = off_of(kt)
                        pt = pts.pop(kt)
                        vtile = v_sb[p * 16 + kt]
                        for b in (b0, b1):
                            blo = (b - b0) * 512
                            bhi = blo + 512
                            lo = max(off, blo)
                            if lo >= bhi:
                                continue
                            nc.tensor.matmul(
                                pctx[b][:, lo - blo:],
                                lhsT=vtile[:, nl * 128:(nl + 1) * 128],
                                rhs=pt[:, lo:bhi],
                                start=(kt == 0), stop=(kt == 4 * b + 3),
                            )
                        if kt == 0:
                            nc.vector.tensor_copy(out=dacc, in_=pt)
                        else:
                            nc.vector.tensor_add(dacc[:, off:], dacc[:, off:],
                                                 pt[:, off:])
                        emit_dense(2)

                    st_exp(0)
                    if nkt > 1:
                        st_exp(1)
                    for kt in range(nkt):
                        if kt + 2 < nkt:
                            st_exp(kt + 2)
                        pv_step(kt)
                    # softmax denominator: DVE-accumulated dacc reduced over
                    # its 128 partitions by two cheap PE ones-matmuls (the
                    # pden tile rides the pst pool's PSUM banks)
                    pden = pstp.tile([128, 1024], F32, tag="pst", bufs=2,
                                     name=f"pden{p}{nl}{hf}")
                    for blk in range(2):
                        nc.tensor.matmul(
                            pden[:, blk * 512:(blk + 1) * 512], lhsT=ones128,
                            rhs=dacc[:, blk * 512:(blk + 1) * 512],
                            start=True, stop=True)
                    emit_dense(4)
                    bc = smallp.tile([128, 1024], F32, tag="bc", bufs=2,
                                     name=f"bc{p}{nl}{hf}")
                    nc.vector.reciprocal_approx_fast(out=bc, in_=pden)
                    for b in (b0, b1):
                        nc.vector.tensor_mul(
                            ctx_t[(p, b)][:, nl, :], pctx[b],
                            bc[:, (b - b0) * 512:(b - b0 + 1) * 512])
                    emit_dense(2)

                for p in range(2):
                    for b in range(NSB):
                        ctx_t[(p, b)] = ctxp.tile(
                            [128, 2, 512], BF16, tag=f"ctx{p}{b}",
                            name=f"ctx{p}{b}")
                    for nl in range(2):
                        for hf in range(2):
                            attn_half(p, nl, hf)
                            if nl == 1:
                                # ctx for blocks of this half now complete;
                                # final flush gets split DMAs for a short tail
                                last = p == 1 and hf == 1
                                push_dense(p, 2 * hf, split_dma=last)
                                push_dense(p, 2 * hf + 1, split_dma=last)
                emit_dense(len(dense_fifo))

    nc.finalize()
    return nc


def _host_prep(inputs):
    hs = np.asarray(inputs["hidden_states"], dtype=np.float32)
    alibi = np.asarray(inputs["alibi"], dtype=np.float32)
    w_qkv = np.asarray(inputs["w_qkv"], dtype=np.float32)
    b_qkv = np.asarray(inputs["b_qkv"], dtype=np.float32)
    w_dense = np.asarray(inputs["w_dense"], dtype=np.float32)

    hs_flat = hs.reshape(T, H)
    # hsr[h, p*S + s'] = hs_flat[2 s' + p, h]
    hsr = np.ascontiguousarray(
        hs_flat.reshape(S, 2, H).transpose(2, 1, 0).reshape(H, T))

    # causal template: M[p, x] = 1 if (x - 384) >= p
    xs = np.arange(896, dtype=np.int64)[None, :] - 384
    ps = np.arange(128, dtype=np.int64)[:, None]
    mskt = (xs >= ps).astype(ml_dtypes.bfloat16)

    w3 = w_qkv.reshape(NH, 3 * HD, H)
    b3 = b_qkv.reshape(NH, 3 * HD)
    in_maps = []
    for c in range(8):
        n0, n1 = 2 * c, 2 * c + 1
        wqk = np.concatenate(
            [w3[n0, 0:128], w3[n0, 128:256], w3[n1, 0:128], w3[n1, 128:256]], axis=0)
        wv = np.concatenate([w3[n0, 256:384], w3[n1, 256:384]], axis=0)
        bqk_c = np.concatenate(
            [b3[n0, 0:128], b3[n0, 128:256], b3[n1, 0:128], b3[n1, 128:256]])
        bv_c = np.concatenate([b3[n0, 256:384], b3[n1, 256:384]])
        in_maps.append({
            "hsr": hsr.astype(ml_dtypes.bfloat16),
            "wqkT": np.ascontiguousarray(wqk.T).astype(ml_dtypes.bfloat16),
            "wvT": np.ascontiguousarray(wv.T).astype(ml_dtypes.bfloat16),
            "wdT": np.ascontiguousarray(w_dense[:, 256 * c:256 * (c + 1)].T).astype(ml_dtypes.bfloat16),
            "bqk": np.ascontiguousarray(bqk_c),
            "bvbc": np.ascontiguousarray(np.tile(bv_c[None, :], (128, 1))),
            "albt": np.ascontiguousarray(
                alibi[[n0, n1], 0, :].reshape(2, NKT, 128).transpose(2, 0, 1)),
            "mskt": mskt,
        })
    return in_maps


def run(inputs, trace=False):
    if "nc" not in _cache:
        _cache["nc"] = _build_nc()
    nc = _cache["nc"]
    in_maps = _host_prep(inputs)
    res = run_bass_kernel_spmd(nc, in_maps, list(range(8)), trace=trace)
    _cache["last_res"] = res
    b_dense = np.asarray(inputs["b_dense"], dtype=np.float32)
    acc = res.results[0]["part"].astype(np.float32)
    for i in range(1, 8):
        acc = acc + res.results[i]["part"].astype(np.float32)
    out = (acc + b_dense[None, :]).reshape(B, S, H)
    return out, res.exec_time_ns


def kernel(**inputs):
    # First execution after a fresh NEFF compile has been observed to flake
    # once; run twice and return the second result.
    run(inputs, trace=False)
    out, _ = run(inputs, trace=False)
    return out


# revision 12
# speedup vs baseline: 1.1840x; 1.1840x over previous
"""BloomAttention Trainium2 kernel (v2).

Reference semantics (B=2, S=2048, H=2048, NH=16, HD=128):
  mixed = hs @ w_qkv.T + b_qkv, reshaped [b,s,nh,3hd] then reinterpreted
  Megatron-style as (s, b*nh, hd).  With B=2 that reinterpretation scrambles
  (batch, position) into 32 independent "virtual sequences" indexed by
  (parity p, head n): virtual seq (p, n) consists of flat tokens
  t = 2*s' + p (t = b*S + s_pos) in increasing s' order.  Attention (with
  alibi[n, k'] bias, causal mask over virtual positions, softmax) runs per
  virtual sequence; the dense projection maps back so that
  out[p, s', :] = dense(concat_n ctx_{p,n}[s']).

Sharding: 2 heads per core (Megatron column-split of w_qkv, row-split of
w_dense), both parities; host sums the 8 partial dense outputs.

Device layouts (per core c, heads {2c, 2c+1}):
  hsr  [2048h, 4096t']   t' = p*2048 + s'  (host pre-transposed/reordered)
  qk   [128j, 1024t']    packed per (jt, p, half) strips; j = q/k head dims
  v    [4096t', 256c']   c' = (n_l, d)
  scores S.T [k', q'] strips of 1024 queries; P = exp(S/sqrt(HD)+alibi)*tri
  den: DVE bf16 tile-accumulate over k-tiles + GpSimd partition_all_reduce
  ctx.T [128d, q'] accumulated in PSUM per 512-query block
  dense: lagged one block, matmuls interleaved into the attention PE
  stream; PSUM->SBUF copies rotate DVE/Pool/Scalar; DMA out on sync.
"""

import math
import os
import sys

for _p in ("/opt/trn_rl_repo", "/root/.axon_site/_ro/trn_rl_repo"):
    if os.path.isdir(_p) and _p not in sys.path:
        sys.path.append(_p)

import numpy as np
import ml_dtypes
import concourse.bass as bass
import concourse.tile as tile
from concourse import mybir, bacc, bass_isa
from concourse.bass_utils import run_bass_kernel_spmd

F32 = mybir.dt.float32
BF16 = mybir.dt.bfloat16
AF = mybir.ActivationFunctionType

B, S, H, NH = 2, 2048, 2048, 16
HD = H // NH
T = B * S                  # 4096 flat tokens
NHT = H // 128             # 16 h-tiles
JQK = 4 * 128              # local q+k rows
JV = 2 * 128               # local v rows
NTB = T // 512             # 8 token-blocks
NKT = S // 128             # 16 key tiles per virtual sequence
NSB = S // 512             # 4 query blocks per virtual sequence
INV_SQRT_HD = 1.0 / math.sqrt(HD)

_cache = {}


def _build_nc():
    nc = bacc.Bacc()
    hsr = nc.declare_dram_parameter("hsr", [H, T], BF16, isOutput=False)
    wqkT = nc.declare_dram_parameter("wqkT", [H, JQK], BF16, isOutput=False)
    wvT = nc.declare_dram_parameter("wvT", [H, JV], BF16, isOutput=False)
    wdT = nc.declare_dram_parameter("wdT", [JV, H], BF16, isOutput=False)
    bqk = nc.declare_dram_parameter("bqk", [JQK], F32, isOutput=False)
    bvbc = nc.declare_dram_parameter("bvbc", [128, JV], F32, isOutput=False)
    albt = nc.declare_dram_parameter("albt", [128, 2, NKT], F32, isOutput=False)
    mskt = nc.declare_dram_parameter("mskt", [128, 896], BF16, isOutput=False)
    part = nc.declare_dram_parameter("part", [T, H], BF16, isOutput=True)

    with tile.TileContext(nc) as tc:
        with (
            tc.tile_pool(name="consts", bufs=1) as consts,
            tc.tile_pool(name="qkvout", bufs=1) as qkvout,
        ):
            # ---------------- Phase A: front-load DMAs ----------------
            # Emission order matters: the first matmul needs wqk(hg0,j0) and
            # hsr(tb0,hg0,j0) only, so those go first on their queues.
            with tc.tile_pool(name="wpool", bufs=1) as wpool:
                wqk_big = []
                wv_big = []
                for hg in range(4):
                    wqk_big.append(wpool.tile([128, 4, JQK], BF16,
                                              tag=f"wqk{hg}", name=f"wqk{hg}"))
                    wv_big.append(wpool.tile([128, 4, JV], BF16,
                                             tag=f"wv{hg}", name=f"wv{hg}"))
                # wqk hg0 split by j for lowest first-tile latency (sync
                # queue); j0 further split in two for parallel DMA engines
                nc.sync.dma_start(out=wqk_big[0][:, 0, 0:256],
                                  in_=wqkT[0:128, 0:256])
                nc.sync.dma_start(out=wqk_big[0][:, 0, 256:512],
                                  in_=wqkT[0:128, 256:512])
                for j in range(1, 4):
                    nc.sync.dma_start(
                        out=wqk_big[0][:, j, :],
                        in_=wqkT[j * 128:(j + 1) * 128, :])
                # hg1 on sync; hg2/hg3 on scalar queue
                nc.sync.dma_start(
                    out=wqk_big[1],
                    in_=wqkT[512:1024, :].rearrange("(j p) f -> p j f", p=128))
                for hg in (2, 3):
                    nc.scalar.dma_start(
                        out=wqk_big[hg],
                        in_=wqkT[hg * 512:(hg + 1) * 512, :].rearrange(
                            "(j p) f -> p j f", p=128))
                # wv split across sync/scalar queues
                for hg in range(4):
                    eng = nc.sync if hg % 2 == 0 else nc.scalar
                    eng.dma_start(
                        out=wv_big[hg],
                        in_=wvT[hg * 512:(hg + 1) * 512, :].rearrange(
                            "(j p) f -> p j f", p=128))
                # consts + dense weight, after the critical weights
                bqk_sb = consts.tile([128, 4], F32)
                nc.sync.dma_start(out=bqk_sb,
                                  in_=bqk.rearrange("(jt p) -> p jt", p=128))
                mask_sb = consts.tile([128, 896], BF16)
                nc.sync.dma_start(out=mask_sb, in_=mskt[:, :])
                tri128 = mask_sb[:, 384:512]
                ones128 = mask_sb[:, 512:640]
                bv_bc = consts.tile([128, JV], F32)
                nc.scalar.dma_start(out=bv_bc, in_=bvbc[:, :])
                alb_sb = consts.tile([128, 2, NKT], F32)
                nc.sync.dma_start(out=alb_sb, in_=albt[:, :, :])
                wd_sb = consts.tile([128, 2, H], BF16)
                for nl in range(2):
                    nc.scalar.dma_start(out=wd_sb[:, nl, :],
                                        in_=wdT[nl * 128:(nl + 1) * 128, :])

                def wqk_t(ht):
                    return wqk_big[ht // 4][:, ht % 4, :]

                def wv_tt(ht):
                    return wv_big[ht // 4][:, ht % 4, :]

                # qk strips packed per (jt, p, half): [128, 1024]
                qk_sb = {}
                for jt in range(4):
                    for p in range(2):
                        for hf in range(2):
                            qk_sb[(jt, p, hf)] = qkvout.tile(
                                [128, 1024], BF16, tag=f"qk{jt}_{p}{hf}",
                                name=f"qk{jt}_{p}{hf}")
                v_sb = {}   # tt -> [128, 256] tile

                # ---------------- Phase B: QKV projection ----------------
                with (
                    tc.tile_pool(name="hsrp", bufs=1) as hsrp,
                    tc.tile_pool(name="pqk", bufs=1, space="PSUM") as pqk,
                    tc.tile_pool(name="pvp", bufs=1, space="PSUM") as pvp,
                ):
                    for tb in range(NTB):
                        hbig = []
                        for hg in range(4):
                            h_t = hsrp.tile([128, 4, 512], BF16, tag="hsr",
                                            bufs=8, name=f"hsr{tb}_{hg}")
                            if tb == 0 and hg == 0:
                                nc.gpsimd.dma_start(out=h_t[:, 0, 0:256],
                                                    in_=hsr[0:128, 0:256])
                                nc.gpsimd.dma_start(out=h_t[:, 0, 256:512],
                                                    in_=hsr[0:128, 256:512])
                                for j in range(1, 4):
                                    nc.gpsimd.dma_start(
                                        out=h_t[:, j, :],
                                        in_=hsr[j * 128:(j + 1) * 128, 0:512])
                            else:
                                nc.gpsimd.dma_start(
                                    out=h_t,
                                    in_=hsr[hg * 512:(hg + 1) * 512,
                                            tb * 512:(tb + 1) * 512].rearrange(
                                                "(j p) f -> p j f", p=128))
                            hbig.append(h_t)

                        def hs_t(ht):
                            return hbig[ht // 4][:, ht % 4, :]

                        pq = [pqk.tile([128, 512], F32, tag=f"pq{jt}",
                                       name=f"pq{jt}_{tb}") for jt in range(4)]
                        pv = [pvp.tile([128, JV], F32, tag=f"pv{tt}",
                                       name=f"pv{tt}_{tb}") for tt in range(4)]
                        for ht in range(NHT):
                            st = ht == 0
                            sp = ht == NHT - 1
                            for jt in range(4):
                                nc.tensor.matmul(
                                    pq[jt],
                                    lhsT=wqk_t(ht)[:, jt * 128:(jt + 1) * 128],
                                    rhs=hs_t(ht),
                                    start=st, stop=sp,
                                )
                        for ht in range(NHT):
                            st = ht == 0
                            sp = ht == NHT - 1
                            for tt in range(4):
                                nc.tensor.matmul(
                                    pv[tt],
                                    lhsT=hs_t(ht)[:, tt * 128:(tt + 1) * 128],
                                    rhs=wv_tt(ht),
                                    start=st, stop=sp,
                                )
                        p_of, i_of = tb // 4, tb % 4
                        hf_of, sl_of = i_of // 2, i_of % 2
                        for jt in range(4):
                            nc.vector.tensor_scalar_add(
                                qk_sb[(jt, p_of, hf_of)][:, sl_of * 512:
                                                         (sl_of + 1) * 512],
                                pq[jt], bqk_sb[:, jt:jt + 1])
                        for tt in range(4):
                            vt = qkvout.tile([128, JV], BF16,
                                             tag=f"v{tb * 4 + tt}",
                                             name=f"v{tb * 4 + tt}")
                            nc.vector.tensor_add(vt, pv[tt], bv_bc)
                            v_sb[tb * 4 + tt] = vt

            # ---------------- Phase C: attention + dense ----------------
            with (
                tc.tile_pool(name="ctxp", bufs=1) as ctxp,
                tc.tile_pool(name="ptp", bufs=1) as ptp,
                tc.tile_pool(name="daccp", bufs=1) as daccp,
                tc.tile_pool(name="smallp", bufs=1) as smallp,
                tc.tile_pool(name="outsbp", bufs=1) as outsbp,
                tc.tile_pool(name="pstp", bufs=1, space="PSUM") as pstp,
                tc.tile_pool(name="pctxp", bufs=1, space="PSUM") as pctxp,
                tc.tile_pool(name="poutp", bufs=1, space="PSUM") as poutp,
            ):
                ctx_t = {}  # (p, b) -> [128, 2, 512] tile

                # ---- lagged dense machinery ----
                dense_fifo = []
                # copy-engine rotation: DVE x9, Scalar x7 per block
                # (GPSIMD cannot read PSUM)
                COPY_PAT = "VSVSVVSVSVVSVSVS"
                # out-DMA issue queue rotation: sync x10, gpsimd x6
                DMAQ_PAT = "YGYYGYYGYYGYYGYG"

                def push_dense(p, b, split_dma=False):
                    ct = ctx_t[(p, b)]
                    for idx in range(16):
                        i, hb = idx // 4, idx % 4
                        eng = COPY_PAT[idx]
                        dq = DMAQ_PAT[idx]

                        def chunk(i=i, hb=hb, eng=eng, dq=dq, ct=ct, p=p, b=b):
                            tt = p * 16 + b * 4 + i
                            po = poutp.tile([128, 512], F32, tag="pout",
                                            bufs=2, name=f"po{tt}_{hb}")
                            for nl in range(2):
                                nc.tensor.matmul(
                                    po,
                                    lhsT=ct[:, nl, i * 128:(i + 1) * 128],
                                    rhs=wd_sb[:, nl, hb * 512:(hb + 1) * 512],
                                    start=(nl == 0), stop=(nl == 1),
                                )
                            ot = outsbp.tile([128, 512], BF16, tag="outsb",
                                             bufs=12, name=f"ot{tt}_{hb}")
                            if eng == "V":
                                nc.vector.tensor_copy(out=ot, in_=po)
                            else:
                                nc.scalar.copy(out=ot, in_=po)
                            dst = part[tt * 128:(tt + 1) * 128,
                                       hb * 512:(hb + 1) * 512]
                            if split_dma:
                                nc.sync.dma_start(out=dst[:, 0:256],
                                                  in_=ot[:, 0:256])
                                nc.gpsimd.dma_start(out=dst[:, 256:512],
                                                    in_=ot[:, 256:512])
                            elif dq == "Y":
                                nc.sync.dma_start(out=dst, in_=ot)
                            else:
                                nc.gpsimd.dma_start(out=dst, in_=ot)
                        dense_fifo.append(chunk)

                def emit_dense(n):
                    for _ in range(min(n, len(dense_fifo))):
                        dense_fifo.pop(0)()

                # ---- attention over one (p, nl, half) strip ----
                def attn_half(p, nl, hf):
                    nkt = 8 * hf + 8
                    b0, b1 = 2 * hf, 2 * hf + 1
                    pctx = {
                        b: pctxp.tile([128, 512], F32, tag="pctx", bufs=2,
                                      name=f"pctx{p}{nl}{b}")
                        for b in (b0, b1)
                    }
                    dacc = daccp.tile([128, 1024], BF16, tag="dacc", bufs=2,
                                      name=f"dacc{p}{nl}{hf}")
                    q_rhs = qk_sb[(2 * nl, p, hf)]
                    kq = qk_sb[(2 * nl + 1, p, 0)], qk_sb[(2 * nl + 1, p, 1)]
                    pts = {}

                    def off_of(kt):
                        return max(0, 128 * kt - 1024 * hf)

                    def st_exp(kt):
                        off = off_of(kt)
                        pst = pstp.tile([128, 1024], F32, tag="pst", bufs=2,
                                        name=f"pst{p}{nl}{hf}_{kt}")
                        ktile = kq[kt // 8][:, (kt % 8) * 128:(kt % 8 + 1) * 128]
                        for blk in range(2):
                            lo = max(off, blk * 512)
                            hi = (blk + 1) * 512
                            if lo >= hi:
                                continue
                            nc.tensor.matmul(
                                pst[:, lo:hi], lhsT=ktile, rhs=q_rhs[:, lo:hi],
                                start=True, stop=True,
                            )
                        pt = ptp.tile([128, 1024], BF16, tag="pt", bufs=4,
                                      name=f"pt{p}{nl}{hf}_{kt}")
                        nc.scalar.activation(out=pt[:, off:], in_=pst[:, off:],
                                             func=AF.Exp,
                                             bias=alb_sb[:, nl, kt:kt + 1],
                                             scale=INV_SQRT_HD)
                        if kt >= 8 * hf:  # diagonal k-tile for this half
                            nc.vector.tensor_mul(
                                pt[:, off:off + 128], pt[:, off:off + 128],
                                tri128)
                        pts[kt] = pt

                    def pv_step(kt):
                        off = off_of(kt)
                        pt = pts.pop(kt)
                        vtile = v_sb[p * 16 + kt]
                        for b in (b0, b1):
                            blo = (b - b0) * 512
                            bhi = blo + 512
                            lo = max(off, blo)
                            if lo >= bhi:
                                continue
                            nc.tensor.matmul(
                                pctx[b][:, lo - blo:],
                                lhsT=vtile[:, nl * 128:(nl + 1) * 128],
                                rhs=pt[:, lo:bhi],
                                start=(kt == 0), stop=(kt == 4 * b + 3),
                            )
                        if kt == 0:
                            nc.vector.tensor_copy(out=dacc, in_=pt)
                        else:
                            nc.vector.tensor_add(dacc[:, off:], dacc[:, off:],
                                                 pt[:, off:])
                        emit_dense(2)

                    st_exp(0)
                    if nkt > 1:
                        st_exp(1)
                    for kt in range(nkt):
                        if kt + 2 < nkt:
                            st_exp(kt + 2)
                        pv_step(kt)
                    # softmax denominator: DVE-accumulated dacc reduced over
                    # its 128 partitions by two cheap PE ones-matmuls (the
                    # pden tile rides the pst pool's PSUM banks)
                    pden = pstp.tile([128, 1024], F32, tag="pst", bufs=2,
                                     name=f"pden{p}{nl}{hf}")
                    for blk in range(2):
                        nc.tensor.matmul(
                            pden[:, blk * 512:(blk + 1) * 512], lhsT=ones128,
                            rhs=dacc[:, blk * 512:(blk + 1) * 512],
                            start=True, stop=True)
                    emit_dense(4)
                    bc = smallp.tile([128, 1024], F32, tag="bc", bufs=2,
                                     name=f"bc{p}{nl}{hf}")
                    nc.vector.reciprocal_approx_fast(out=bc, in_=pden)
                    for b in (b0, b1):
                        nc.vector.tensor_mul(
                            ctx_t[(p, b)][:, nl, :], pctx[b],
                            bc[:, (b - b0) * 512:(b - b0 + 1) * 512])
                    emit_dense(2)

                for p in range(2):
                    for b in range(NSB):
                        ctx_t[(p, b)] = ctxp.tile(
                            [128, 2, 512], BF16, tag=f"ctx{p}{b}",
                            name=f"ctx{p}{b}")
                    for nl in range(2):
                        for hf in range(2):
                            attn_half(p, nl, hf)
                            if nl == 1:
                                # ctx for blocks of this half now complete;
                                # final flush gets split DMAs for a short tail
                                last = p == 1 and hf == 1
                                push_dense(p, 2 * hf, split_dma=last)
                                push_dense(p, 2 * hf + 1, split_dma=last)
                emit_dense(len(dense_fifo))

    nc.finalize()
    return nc


def _host_prep(inputs):
    hs = np.asarray(inputs["hidden_states"], dtype=np.float32)
    alibi = np.asarray(inputs["alibi"], dtype=np.float32)
    w_qkv = np.asarray(inputs["w_qkv"], dtype=np.float32)
    b_qkv = np.asarray(inputs["b_qkv"], dtype=np.float32)
    w_dense = np.asarray(inputs["w_dense"], dtype=np.float32)

    hs_flat = hs.reshape(T, H)
    # hsr[h, p*S + s'] = hs_flat[2 s' + p, h]
    hsr = np.ascontiguousarray(
        hs_flat.reshape(S, 2, H).transpose(2, 1, 0).reshape(H, T))

    # causal template: M[p, x] = 1 if (x - 384) >= p
    xs = np.arange(896, dtype=np.int64)[None, :] - 384
    ps = np.arange(128, dtype=np.int64)[:, None]
    mskt = (xs >= ps).astype(ml_dtypes.bfloat16)

    w3 = w_qkv.reshape(NH, 3 * HD, H)
    b3 = b_qkv.reshape(NH, 3 * HD)
    in_maps = []
    for c in range(8):
        n0, n1 = 2 * c, 2 * c + 1
        wqk = np.concatenate(
            [w3[n0, 0:128], w3[n0, 128:256], w3[n1, 0:128], w3[n1, 128:256]], axis=0)
        wv = np.concatenate([w3[n0, 256:384], w3[n1, 256:384]], axis=0)
        bqk_c = np.concatenate(
            [b3[n0, 0:128], b3[n0, 128:256], b3[n1, 0:128], b3[n1, 128:256]])
        bv_c = np.concatenate([b3[n0, 256:384], b3[n1, 256:384]])
        in_maps.append({
            "hsr": hsr.astype(ml_dtypes.bfloat16),
            "wqkT": np.ascontiguousarray(wqk.T).astype(ml_dtypes.bfloat16),
            "wvT": np.ascontiguousarray(wv.T).astype(ml_dtypes.bfloat16),
            "wdT": np.ascontiguousarray(w_dense[:, 256 * c:256 * (c + 1)].T).astype(ml_dtypes.bfloat16),
            "bqk": np.ascontiguousarray(bqk_c),
            "bvbc": np.ascontiguousarray(np.tile(bv_c[None, :], (128, 1))),
            "albt": np.ascontiguousarray(
                alibi[[n0, n1], 0, :].reshape(2, NKT, 128).transpose(2, 0, 1)),
            "mskt": mskt,
        })
    return in_maps


def run(inputs, trace=False):
    if "nc" not in _cache:
        _cache["nc"] = _build_nc()
    nc = _cache["nc"]
    in_maps = _host_prep(inputs)
    res = run_bass_kernel_spmd(nc, in_maps, list(range(8)), trace=trace)
    _cache["last_res"] = res
    b_dense = np.asarray(inputs["b_dense"], dtype=np.float32)
    acc = res.results[0]["part"].astype(np.float32)
    for i in range(1, 8):
        acc = acc + res.results[i]["part"].astype(np.float32)
    out = (acc + b_dense[None, :]).reshape(B, S, H)
    return out, res.exec_time_ns


def kernel(**inputs):
    # First execution after a fresh NEFF compile has been observed to flake
    # once; run twice and return the second result.
    run(inputs, trace=False)
    out, _ = run(inputs, trace=False)
    return out
